# revision 10
# baseline (speedup 1.0000x reference)
"""Trainium2 Bass kernel for nn_AttentionMechanism (KL-attention teacher distill).

Reference computation (per node n, 8 teachers t, C=H=128):
    x_t   = W1 @ t_t + b1                (teacher logits)       [T,N,H]
    s     = W2 @ s_d + b2                (student logits)       [N,H]
    kl_t  = sum_h softmax(x_t) * (log_softmax(x_t) - log_softmax(s))
    w     = softmax_t(-kl_t / sqrt(128))
    y     = sum_t w_t * t_t

Key identities used on device:
    kl_t = D_t/Z_t - log Z_t + log Zs,  Z_t = sum_h exp(x_t),
    D_t  = sum_h exp(x_t) * (x_t - s).
    log Zs is constant over t -> drops out of the softmax over t.
So the student branch only contributes s (un-normalized logits), never a
softmax. Scores g_t = log Z_t - D_t/Z_t, weights = softmax_t(g_t/sqrt(128)).

Device layout: score path runs transposed ([h, n] tiles, reductions over h
via selector-matmuls on the PE); value path runs natural ([n, c] tiles,
per-partition tensor_scalar muls + fp32r identity-matmul accumulation in
PSUM). The host feeds t both ways: natural fp32 and transposed bf16.

Sharding: node dimension split across 8 cores (data parallel), weights
replicated; no collectives.
"""

import math
import os
import numpy as np

T_MODELS = 8
N_NODES = 100000
C_IN = 128
H_HID = 128
N_CORES = 8
NT = 512                      # nodes per on-device tile
SUB = NT // 128               # 128-node subtiles per tile
N_SHARD = N_NODES // N_CORES  # 12500
N_PAD = ((N_SHARD + NT - 1) // NT) * NT   # 12800
N_TILES = N_PAD // NT         # 25

# split of the 32 per-tile value-path muls across engines
VMUL_ENGINES = ("vector",) * 14 + ("scalar",) * 6 + ("gpsimd",) * 12


def build_program(n_pad=N_PAD):
    from contextlib import ExitStack
    import concourse.bacc as bacc
    import concourse.tile as tile
    from concourse import mybir

    f32 = mybir.dt.float32
    f32r = mybir.dt.float32r
    bf16 = mybir.dt.bfloat16
    AF = mybir.ActivationFunctionType
    OP = mybir.AluOpType
    n_tiles = n_pad // NT

    nc = bacc.Bacc()

    # ---- DRAM parameters (names = in_map keys) ----
    t_nat = nc.declare_dram_parameter(
        "t_nat", [n_pad // NT, 128, T_MODELS, SUB, C_IN], f32, isOutput=False)
    tT16 = nc.declare_dram_parameter(
        "tT16", [n_pad // NT, C_IN, T_MODELS, NT], bf16, isOutput=False)
    sT16 = nc.declare_dram_parameter(
        "sT16", [n_pad // NT, C_IN, NT], bf16, isOutput=False)
    w1T16_p = nc.declare_dram_parameter("w1T16", [C_IN, H_HID], bf16, isOutput=False)
    w2T16n_p = nc.declare_dram_parameter("w2T16n", [C_IN, H_HID], bf16, isOutput=False)
    b1c_p = nc.declare_dram_parameter("b1c", [H_HID, 1], f32, isOutput=False)
    bd16_p = nc.declare_dram_parameter("bd16", [1, H_HID], bf16, isOutput=False)
    ones16_p = nc.declare_dram_parameter("ones16", [1, NT], bf16, isOutput=False)
    sel16_p = nc.declare_dram_parameter("sel16", [H_HID, 16, 16], bf16, isOutput=False)
    id16_p = nc.declare_dram_parameter("id16", [128, 128], bf16, isOutput=False)
    id32_p = nc.declare_dram_parameter("id32", [128, 128], f32r, isOutput=False)
    idT32_p = nc.declare_dram_parameter("idT32", [16, 16], f32, isOutput=False)
    y_out = nc.declare_dram_parameter("y", [n_pad, C_IN], f32, isOutput=True)

    inv_sqrt_d = 1.0 / math.sqrt(float(C_IN))

    with ExitStack() as ctx:
        tc = ctx.enter_context(tile.TileContext(nc))
        singles = ctx.enter_context(tc.tile_pool(name="singles", bufs=1))
        big = ctx.enter_context(tc.tile_pool(name="big", bufs=2))
        work = ctx.enter_context(tc.tile_pool(name="work", bufs=3))
        tmpp = ctx.enter_context(tc.tile_pool(name="tmpp", bufs=3))
        smal = ctx.enter_context(tc.tile_pool(name="smal", bufs=2))
        outp = ctx.enter_context(tc.tile_pool(name="outp", bufs=2))
        ps_p_pool = ctx.enter_context(tc.tile_pool(name="psP", bufs=2, space="PSUM"))
        ps_s_pool = ctx.enter_context(tc.tile_pool(name="psS", bufs=2, space="PSUM"))
        ps_y_pool = ctx.enter_context(tc.tile_pool(name="psY", bufs=2, space="PSUM"))

        # ---- load constants once ----
        sb_w1T = singles.tile([C_IN, H_HID], bf16)
        nc.sync.dma_start(out=sb_w1T, in_=w1T16_p[:, :])
        sb_w2Tn = singles.tile([C_IN, H_HID], bf16)
        nc.sync.dma_start(out=sb_w2Tn, in_=w2T16n_p[:, :])
        sb_b1c = singles.tile([H_HID, 1], f32)
        nc.sync.dma_start(out=sb_b1c, in_=b1c_p[:, :])
        sb_bd = singles.tile([1, H_HID], bf16)
        nc.sync.dma_start(out=sb_bd, in_=bd16_p[:, :])
        sb_ones = singles.tile([1, NT], bf16)
        nc.sync.dma_start(out=sb_ones, in_=ones16_p[:, :])
        sb_sel = singles.tile([H_HID, 16, 16], bf16)
        nc.sync.dma_start(out=sb_sel, in_=sel16_p[:, :, :])
        sb_id16 = singles.tile([128, 128], bf16)
        nc.sync.dma_start(out=sb_id16, in_=id16_p[:, :])
        sb_id32 = singles.tile([128, 128], f32r)
        nc.sync.dma_start(out=sb_id32, in_=id32_p[:, :])
        sb_idT = singles.tile([16, 16], f32)
        nc.sync.dma_start(out=sb_idT, in_=idT32_p[:, :])

        # warm the ACT table set (Exp+Ln together -> natural_log_exp set)
        # so no mid-stream activation carries a table load (wait-slot limit)
        warm_i = singles.tile([128, 1], f32)
        nc.vector.memset(warm_i, 1.0)
        warm_o = singles.tile([128, 1], f32)
        nc.scalar.activation(warm_o, warm_i, AF.Exp)
        nc.scalar.activation(warm_o, warm_i, AF.Ln)

        for i in range(n_tiles):
            n0 = i * NT
            # ---- loads ----
            tT_t = big.tile([C_IN, T_MODELS, NT], bf16, tag="tT")
            nc.sync.dma_start(out=tT_t, in_=tT16[i])
            tn_t = big.tile([128, T_MODELS, SUB, C_IN], f32, tag="tnat")
            nc.sync.dma_start(out=tn_t, in_=t_nat[i])
            sT_t = big.tile([C_IN, NT], bf16, tag="sT")
            nc.sync.dma_start(out=sT_t, in_=sT16[i])

            # ---- student branch: negss = -(W2 s) + (b1 - b2), bf16 in SBUF ----
            ps_s = ps_s_pool.tile([H_HID, NT], f32, tag="ps_misc")
            nc.tensor.matmul(ps_s, lhsT=sb_w2Tn, rhs=sT_t, start=True, stop=False)
            nc.tensor.matmul(ps_s, lhsT=sb_bd, rhs=sb_ones, start=False, stop=True)
            negss = outp.tile([H_HID, NT], bf16, tag="negss")
            nc.scalar.copy(negss, ps_s)

            # ---- teacher loop: scores ----
            ps_stats = ps_s_pool.tile([16, NT], f32, tag="ps_stats")
            ue_list = []
            for t in range(T_MODELS):
                ps_p = ps_p_pool.tile([H_HID, NT], f32, tag="ps_p")
                nc.tensor.matmul(
                    ps_p, lhsT=sb_w1T, rhs=tT_t[:, t, :], start=True, stop=True
                )
                ue = work.tile([H_HID, 2, NT], bf16, tag="ue")
                # u = exp(x + b1)   (b1 per-partition over h)
                nc.scalar.activation(
                    ue[:, 0, :], ps_p, AF.Exp, bias=sb_b1c, scale=1.0
                )
                # psum becomes d = x + negss = (x+b1) - (W2 s + b2)
                nc.tensor.matmul(
                    ps_p, lhsT=sb_id16, rhs=negss, start=False, stop=True,
                    skip_group_check=True,
                )
                # e = u * d
                nc.vector.tensor_mul(ue[:, 1, :], ue[:, 0, :], ps_p)
                # Z_t and D_t rows via selector matmuls
                nc.tensor.matmul(
                    ps_stats, lhsT=sb_sel[:, t, :], rhs=ue[:, 0, :],
                    start=(t == 0), stop=False, skip_group_check=True,
                )
                nc.tensor.matmul(
                    ps_stats, lhsT=sb_sel[:, 8 + t, :], rhs=ue[:, 1, :],
                    start=False, stop=(t == T_MODELS - 1), skip_group_check=True,
                )
                ue_list.append(ue)

            # ---- stats -> per-node layout ----
            stats32 = smal.tile([16, NT], f32, tag="stats32")
            nc.vector.tensor_copy(stats32, ps_stats)
            ps_T = ps_s_pool.tile([128, SUB * 16], f32, tag="ps_misc")
            for s in range(SUB):
                nc.tensor.transpose(
                    ps_T[:, s * 16:(s + 1) * 16],
                    stats32[:, s * 128:(s + 1) * 128],
                    sb_idT,
                )
            sT32 = smal.tile([128, SUB, 16], f32, tag="sT32")
            nc.vector.tensor_copy(sT32, ps_T.rearrange("p (s q) -> p s q", q=16))
            Z = sT32[:, :, 0:8]
            D = sT32[:, :, 8:16]

            R = smal.tile([128, SUB, 8], f32, tag="R")
            nc.vector.reciprocal(R, Z)
            L = smal.tile([128, SUB, 8], f32, tag="L")
            nc.scalar.activation(L, Z, AF.Ln)
            G = smal.tile([128, SUB, 8], f32, tag="G")
            nc.vector.tensor_mul(G, D, R)
            nc.vector.tensor_sub(G, L, G)
            EW = smal.tile([128, SUB, 8], f32, tag="EW")
            nc.scalar.activation(EW, G, AF.Exp, scale=inv_sqrt_d)
            S = smal.tile([128, SUB, 1], f32, tag="S")
            nc.vector.tensor_reduce(S, EW, axis=mybir.AxisListType.X, op=OP.add)
            RS = smal.tile([128, SUB, 1], f32, tag="RS")
            nc.vector.reciprocal(RS, S)
            W = smal.tile([128, SUB, 8], f32, tag="W")
            nc.vector.tensor_mul(W, EW, RS.to_broadcast([128, SUB, 8]))

            # ---- value path: y = sum_t w_t * t_t ----
            ps_y = ps_y_pool.tile([128, SUB * C_IN], f32, tag="ps_y")
            eng_i = 0
            for t in range(T_MODELS):
                tmp = tmpp.tile([128, SUB, C_IN], f32r, tag="tmp")
                for s in range(SUB):
                    eng = VMUL_ENGINES[eng_i % len(VMUL_ENGINES)]
                    eng_i += 1
                    w_ap = W[:, s, t:t + 1]
                    if eng == "vector":
                        nc.vector.tensor_scalar_mul(
                            tmp[:, s, :], tn_t[:, t, s, :], w_ap
                        )
                    elif eng == "scalar":
                        nc.scalar.mul(tmp[:, s, :], tn_t[:, t, s, :], w_ap)
                    else:
                        nc.gpsimd.tensor_scalar_mul(
                            tmp[:, s, :], tn_t[:, t, s, :], w_ap
                        )
                nc.tensor.matmul(
                    ps_y,
                    lhsT=sb_id32,
                    rhs=tmp.rearrange("p s c -> p (s c)"),
                    start=(t == 0), stop=(t == T_MODELS - 1),
                    skip_group_check=True,
                )

            y32 = outp.tile([128, SUB, C_IN], f32, tag="y32")
            nc.scalar.copy(y32, ps_y.rearrange("p (s c) -> p s c", c=C_IN))
            nc.sync.dma_start(
                out=y_out[n0:n0 + NT, :].rearrange("(s p) c -> p s c", p=128),
                in_=y32,
            )

    nc.finalize()
    return nc


def _prep_host_inputs(s_output, t_output, w1_w, w1_b, w2_w, w2_b, n_pad=N_PAD,
                      n_cores=N_CORES):
    """Shard + lay out host-side arrays. Returns list of per-core in_maps."""
    import ml_dtypes

    bf = ml_dtypes.bfloat16
    f32 = np.float32
    t_output = np.asarray(t_output, dtype=f32)
    s_output = np.asarray(s_output, dtype=f32)
    w1_w = np.asarray(w1_w, dtype=f32)
    w1_b = np.asarray(w1_b, dtype=f32)
    w2_w = np.asarray(w2_w, dtype=f32)
    w2_b = np.asarray(w2_b, dtype=f32)

    n_shard = t_output.shape[1] // n_cores

    # constants (identical on every core)
    sel = np.zeros((H_HID, 16, 16), dtype=bf)
    for r in range(16):
        sel[:, r, r] = 1.0
    consts = {
        "w1T16": np.ascontiguousarray(w1_w.T).astype(bf),
        "w2T16n": np.ascontiguousarray(-w2_w.T).astype(bf),
        "b1c": np.ascontiguousarray(w1_b.reshape(H_HID, 1)),
        "bd16": np.ascontiguousarray((w1_b - w2_b).reshape(1, H_HID)).astype(bf),
        "ones16": np.ones((1, NT), dtype=bf),
        "sel16": sel,
        "id16": np.eye(128, dtype=f32).astype(bf),
        "id32": np.eye(128, dtype=f32),
        "idT32": np.eye(16, dtype=f32),
    }

    in_maps = []
    for c in range(n_cores):
        sl = slice(c * n_shard, (c + 1) * n_shard)
        t_sh = t_output[:, sl, :]                      # [T, n_shard, C]
        s_sh = s_output[sl, :]                         # [n_shard, C]
        t_pad = np.zeros((T_MODELS, n_pad, C_IN), dtype=f32)
        t_pad[:, :n_shard, :] = t_sh
        s_pad = np.zeros((n_pad, C_IN), dtype=f32)
        s_pad[:n_shard, :] = s_sh
        ntl = n_pad // NT
        # device-order marshaling: each tile's load is one contiguous block
        t_dev = np.ascontiguousarray(
            t_pad.reshape(T_MODELS, ntl, SUB, 128, C_IN).transpose(1, 3, 0, 2, 4))
        tT_dev = np.ascontiguousarray(
            t_pad.transpose(0, 2, 1).reshape(T_MODELS, C_IN, ntl, NT)
            .transpose(2, 1, 0, 3)).astype(bf)
        sT_dev = np.ascontiguousarray(
            s_pad.T.reshape(C_IN, ntl, NT).transpose(1, 0, 2)).astype(bf)
        m = {
            "t_nat": t_dev,
            "tT16": tT_dev,
            "sT16": sT_dev,
        }
        m.update(consts)
        in_maps.append(m)
    return in_maps, n_shard


def kernel(s_output, t_output, w1_w, w1_b, w2_w, w2_b):
    from concourse.bass_utils import run_bass_kernel_spmd

    in_maps, n_shard = _prep_host_inputs(
        s_output, t_output, w1_w, w1_b, w2_w, w2_b
    )
    nc = build_program(N_PAD)
    res = run_bass_kernel_spmd(
        nc, in_maps, list(range(N_CORES)),
        trace=bool(int(os.environ.get("KERNEL_TRACE", "0"))),
    )
    outs = [np.asarray(r["y"][:n_shard], dtype=np.float32) for r in res.results]
    return np.concatenate(outs, axis=0)
